# revision 8
# baseline (speedup 1.0000x reference)
"""Transformer block (LN -> MHA -> residual -> LN -> MLP -> residual) on 8 TRN2
NeuronCores.

Sharding: pure row data-parallelism over (batch, sequence-half). Core c handles
batch b = c//2 and query rows [h*512, (h+1)*512) with h = c%2. Each core
computes K/V projections for its full batch locally (small duplicated work),
which removes every cross-core collective. Host reorders each core's batch rows
"own rows first" so the same SPMD program works on all cores; mask columns are
permuted identically (softmax/attention are permutation-invariant over keys).

v3: DMA/queue rebalance + bubble removal on top of v2's fp8 attention.
  - x tiles stream on 4 DMA queues (sync/vector/gpsimd/tensor) instead of one,
    so LN1 is fed 4x faster and stage A becomes PE-bound almost immediately.
  - all Q/K weights issue up front on the gpsimd queue; pool-slot rotation
    (bufs=4 per tag) paces them one head ahead of compute automatically.
  - wo preloads on the tensor queue behind the x tiles; w1 streams on
    scalar+sync, w2 on gpsimd+vector; output stores rotate over queues.
  - residual r stays fully in SBUF (no DRAM bounce); the f32 x rows load into
    the same tile during attention and the O-proj adds in place.
  - MLP-up's first 3 weight chunks run as two FD-256 halves (gated on qt01 /
    qt23) so the PE starts the MLP while the last LN2 tiles are finishing.
  - gelu activation table preloaded during stage A (off the D->F critical
    path); stage-D xn2T transposes on the otherwise-idle gpsimd engine.
fp32 PSUM accumulation everywhere; statistics stay fp32.
"""

import numpy as np
import ml_dtypes

import concourse.bass as bass
import concourse.tile as tile
from concourse import bacc, mybir
from concourse.bass_utils import run_bass_kernel_spmd

BF16 = mybir.dt.bfloat16
F32 = mybir.dt.float32
FP8 = mybir.dt.float8e4
AX = mybir.AxisListType
OP = mybir.AluOpType
ACT = mybir.ActivationFunctionType
DR = mybir.MatmulPerfMode.DoubleRow

P = 128
B, T, C, H = 4, 1024, 2048, 4
DH = C // H                      # 512
F = 4 * C                        # 8192
R = T // 2                       # 512 own query rows per core
RT, TT, CT, FT = R // P, T // P, C // P, F // P   # 4, 8, 16, 64
CP = CT // 2                     # 8 double-row contraction steps over C
HT = DH // P                     # 4 feature tiles per head
EPS = 1e-5
ISQ = 1.0 / float(np.sqrt(DH))
NEGBIG = 30000.0


def _bcast_load(nc, pool, dram_ap, name, dtype):
    """Broadcast a [n] DRAM vector to all 128 partitions -> [128, n]."""
    t = pool.tile([P, dram_ap.shape[0]], dtype, name=name)
    src = bass.AP(
        tensor=dram_ap.tensor, offset=dram_ap.offset, ap=[[0, P]] + list(dram_ap.ap)
    )
    nc.gpsimd.dma_start(out=t[:], in_=src)
    return t


def _ln_tile(nc, pool, x_sl, eps_t, tag, i):
    """Normalize one [128, C] tile -> bf16 (x-mu)*rstd. The LN affine (w,b)
    is folded into the following matmul's weights/biases on the host."""
    stats = pool.tile([P, 4, 6], F32, name=f"{tag}_stats{i}", tag=f"{tag}_stats",
                      bufs=2)
    for sg in range(4):
        nc.vector.bn_stats(out=stats[:, sg, :], in_=x_sl[:, sg * 512:(sg + 1) * 512])
    mv = pool.tile([P, 2], F32, name=f"{tag}_mv{i}", tag=f"{tag}_mv", bufs=2)
    nc.vector.bn_aggr(out=mv[:], in_=stats[:])
    std = pool.tile([P, 1], F32, name=f"{tag}_std{i}", tag=f"{tag}_std", bufs=2)
    nc.scalar.activation(out=std[:], in_=mv[:, 1:2], func=ACT.Sqrt,
                         bias=eps_t[:], scale=1.0)
    rstd = pool.tile([P, 1], F32, name=f"{tag}_rstd{i}", tag=f"{tag}_rstd", bufs=2)
    nc.vector.reciprocal(rstd[:], std[:])
    nmr = pool.tile([P, 1], F32, name=f"{tag}_nmr{i}", tag=f"{tag}_nmr", bufs=2)
    nc.vector.tensor_scalar(nmr[:], mv[:, 0:1], rstd[:], -1.0, OP.mult, OP.mult)
    xh = pool.tile([P, C], BF16, name=f"{tag}_xh{i}", tag=f"{tag}_xh", bufs=1)
    nc.scalar.activation(out=xh[:], in_=x_sl, func=ACT.Identity,
                         bias=nmr[:], scale=rstd[:])
    return xh


def _body(tc):
    nc = tc.nc
    d = {n: nc.dram_tensor(n, s, dt, kind=k).ap() for n, s, dt, k in [
        ("xb", [T, C], BF16, "ExternalInput"),
        ("xq", [R, C], F32, "ExternalInput"),
        ("mask", [R, T], BF16, "ExternalInput"),
        ("wq", [CT, P, CT, P], FP8, "ExternalInput"),
        ("wk", [CT, P, CT, P], FP8, "ExternalInput"),
        ("wv", [P, CT, C], FP8, "ExternalInput"),
        ("wo", [P, CT, 4, 512], FP8, "ExternalInput"),
        ("w1", [FT // 2, P, CT, 2, P], BF16, "ExternalInput"),
        ("w2", [4, FT // 8, P, 8, 512], BF16, "ExternalInput"),
        ("bq", [P, CT], F32, "ExternalInput"),
        ("bk", [P, CT], F32, "ExternalInput"),
        ("b1", [P, FT], F32, "ExternalInput"),
        ("bv", [C], BF16, "ExternalInput"),
        ("bo", [C], BF16, "ExternalInput"),
        ("b2", [C], BF16, "ExternalInput"),
        ("out", [R, C], F32, "ExternalOutput"),
    ]}

    consts = tc.alloc_tile_pool(name="consts", bufs=1)
    eps_t = consts.tile([P, 1], F32, name="eps")
    nc.vector.memset(eps_t[:], EPS)
    # r rows live in SBUF for the whole kernel: loaded with the f32 x rows
    # during attention, O-proj residual adds in place, LN2 + final adds read it.
    p_r = tc.alloc_tile_pool(name="p_r", bufs=1)
    r_sb = p_r.tile([P, RT, C], F32, name="r_sb")
    # right stack: the w1 stream pool at the bottom (lives into stage G);
    # yT/wo above it release once stage D's matmuls retire.
    pW1 = tc.alloc_tile_pool(name="pW1", bufs=1, side="right")
    p_wo = tc.alloc_tile_pool(name="p_wo", bufs=1, side="right")
    wo_t = p_wo.tile([P, CT, 4, 512], FP8, name="wo_t")
    p_yT = tc.alloc_tile_pool(name="p_yT", bufs=1, side="right")
    yT = p_yT.tile([P, CT, R], FP8, name="yT")
    # attention-era pools (all released after AV of the last head)
    p_xnT = tc.alloc_tile_pool(name="p_xnT", bufs=1)
    xnT8_lo = p_xnT.tile([P, CT, R], FP8, name="xnT8_lo")
    xnT8_hi = p_xnT.tile([P, CT, R], FP8, name="xnT8_hi")
    xnT8 = [xnT8_lo, xnT8_hi]
    p_vh = tc.alloc_tile_pool(name="p_vh", bufs=1)
    vh = p_vh.tile([P, TT, C], FP8, name="vh")
    pQKw = tc.alloc_tile_pool(name="pQKw", bufs=1)
    p_mb = tc.alloc_tile_pool(name="p_mb", bufs=1)
    mb = p_mb.tile([P, RT, T], BF16, name="mb")

    # ---------------- Stage A: per-tile LN1 -> transpose -> fp8 cast -> V proj
    p_wv = tc.alloc_tile_pool(name="p_wv", bufs=1)
    wv_t = p_wv.tile([P, CT, C], FP8, name="wv_t")
    lnA = tc.alloc_tile_pool(name="lnA", bufs=1)
    bv_bc = _bcast_load(nc, lnA, d["bv"], "bv_bc", BF16)
    pA = tc.alloc_tile_pool(name="pA", bufs=2)
    psA = tc.alloc_tile_pool(name="psA", bufs=2, space="PSUM")

    # x tiles stream on 2 DMA queues (sync+gpsimd) so LN1 is never DMA-starved;
    # wv streams on the scalar queue in parallel. Only sync/scalar/gpsimd can
    # issue DMAs on this target.
    xt_engs = [nc.sync, nc.gpsimd]
    xts = []
    for tt in range(TT):
        xt = pA.tile([P, C], BF16, name=f"xt{tt}", tag="xt", bufs=2)
        xt_engs[tt % 2].dma_start(xt[:], d["xb"][tt * P:(tt + 1) * P, :])
        xts.append(xt[:])
    for kc in range(4):
        nc.scalar.dma_start(wv_t[:, 4 * kc:4 * (kc + 1), :],
                            d["wv"][:, 4 * kc:4 * (kc + 1), :])
    # wo preload on the sync queue behind x tiles (needed at stage D)
    for kc in range(4):
        nc.sync.dma_start(wo_t[:, 4 * kc:4 * (kc + 1), :, :],
                          d["wo"][:, 4 * kc:4 * (kc + 1), :, :])
    # residual rows (f32) into SBUF behind x tiles, needed at stage D
    nc.sync.dma_start(out=r_sb[:], in_=d["xq"].rearrange("(qo qp) c -> qp qo c", qp=P))
    # gpsimd queue: mask, small constants, then the whole Q/K weight stream
    # (pool-slot rotation paces heads; nothing time-critical sits behind it).
    nc.gpsimd.dma_start(out=mb[:], in_=d["mask"].rearrange("(qo qp) k -> qp qo k", qp=P))
    bq_t = consts.tile([P, CT], F32, name="bq_t")
    nc.gpsimd.dma_start(out=bq_t[:], in_=d["bq"])
    bk_t = consts.tile([P, CT], F32, name="bk_t")
    nc.gpsimd.dma_start(out=bk_t[:], in_=d["bk"])
    b1_t = consts.tile([P, FT], F32, name="b1_t")
    nc.gpsimd.dma_start(out=b1_t[:], in_=d["b1"])
    wqcs, wkcs = {}, {}
    for h in range(H):
        for fl in range(HT):
            fo = h * HT + fl
            wqc = pQKw.tile([P, CT, P], FP8, name=f"wqc{fo}", tag="wqc", bufs=3)
            nc.gpsimd.dma_start(wqc[:], d["wq"][fo])
            wqcs[fo] = wqc
            wkc = pQKw.tile([P, CT, P], FP8, name=f"wkc{fo}", tag="wkc", bufs=3)
            nc.gpsimd.dma_start(wkc[:], d["wk"][fo])
            wkcs[fo] = wkc
    # preload the gelu activation table off the D->F critical path
    gdum = consts.tile([P, 1], F32, name="gdum")
    nc.scalar.activation(out=gdum[:], in_=eps_t[:], func=ACT.Gelu,
                         bias=0.0, scale=1.0)

    # software-pipelined by one tile: cast(tt)+V(tt) are emitted during
    # LN(tt+1) so the fp8 cast's transpose-wait never blocks the next LN
    # apply in the scalar FIFO.
    xnTts = {}

    def _emit_castv(tt):
        half, lt = divmod(tt, 4)
        nc.vector.tensor_copy(xnT8[half][:, :, lt * P:(lt + 1) * P],
                              xnTts.pop(tt)[:])
        for h in range(H):
            ps_v = psA.tile([P, DH], F32, name="ps_v", tag="psA", bufs=2)
            for kp in range(CP):
                nc.tensor.matmul(ps_v[:],
                                 xnT8[half][:, 2 * kp:2 * kp + 2, lt * P:(lt + 1) * P],
                                 wv_t[:, 2 * kp:2 * kp + 2, h * DH:(h + 1) * DH],
                                 start=(kp == 0), stop=(kp == CP - 1), perf_mode=DR)
            nc.vector.tensor_tensor(vh[:, tt, h * DH:(h + 1) * DH], ps_v[:],
                                    bv_bc[:, h * DH:(h + 1) * DH], OP.add)

    for tt in range(TT):
        xn_t = _ln_tile(nc, pA, xts[tt], eps_t, "ln1", tt)
        xnTt = pA.tile([P, CT, P], BF16, name=f"xnTt{tt}", tag="xnTt", bufs=2)
        nc.scalar.dma_start_transpose(xnTt[:], xn_t[:])
        xnTts[tt] = xnTt
        if tt > 0:
            _emit_castv(tt - 1)
    _emit_castv(TT - 1)
    psA.release()
    pA.release()
    lnA.release()
    p_wv.release()

    # mask -> additive bias: 0 where visible, -30000 where masked
    nc.vector.tensor_scalar(mb[:], mb[:], NEGBIG, -NEGBIG, OP.mult, OP.add)

    # ---------------- Stage B+C: software-pipelined per-head Q/K + attention
    pBC = tc.alloc_tile_pool(name="pBC", bufs=2)
    psBC = tc.alloc_tile_pool(name="psBC", bufs=2, space="PSUM")
    hs = {}

    def emit_qk(h):
        qTh = pBC.tile([P, HT, R], FP8, name=f"qTh{h}", tag="qTh", bufs=2)
        kTh = pBC.tile([P, HT, T], FP8, name=f"kTh{h}", tag="kTh", bufs=2)
        for fl in range(HT):
            fo = h * HT + fl
            wqc, wkc = wqcs.pop(fo), wkcs.pop(fo)
            ps_q = psBC.tile([P, R], F32, name="ps_q", tag="psB", bufs=2)
            for kp in range(CP):
                nc.tensor.matmul(ps_q[:], wqc[:, 2 * kp:2 * kp + 2, :],
                                 xnT8_lo[:, 2 * kp:2 * kp + 2, :],
                                 start=(kp == 0), stop=(kp == CP - 1), perf_mode=DR)
            nc.scalar.activation(out=qTh[:, fl, :], in_=ps_q[:], func=ACT.Identity,
                                 bias=bq_t[:, fo:fo + 1], scale=1.0)
            for nn in range(2):
                ps_k = psBC.tile([P, 512], F32, name="ps_k", tag="psB", bufs=2)
                for kp in range(CP):
                    nc.tensor.matmul(ps_k[:], wkc[:, 2 * kp:2 * kp + 2, :],
                                     xnT8[nn][:, 2 * kp:2 * kp + 2, :],
                                     start=(kp == 0), stop=(kp == CP - 1), perf_mode=DR)
                nc.scalar.activation(out=kTh[:, fl, nn * 512:(nn + 1) * 512], in_=ps_k[:],
                                     func=ACT.Identity, bias=bk_t[:, fo:fo + 1],
                                     scale=1.0)
        hs[h] = (qTh, kTh)

    def emit_scores(h):
        qTh, kTh = hs[h]
        attT = pBC.tile([P, TT, R], BF16, name=f"attT{h}", tag="attT", bufs=1)
        attT8 = pBC.tile([P, TT, R], FP8, name=f"attT8{h}", tag="attT8", bufs=2)
        for qt in range(RT):
            ps_s = psBC.tile([P, T], F32, name="ps_s", tag="scores", bufs=2)
            for nn in range(2):
                for dp in range(HT // 2):
                    nc.tensor.matmul(
                        ps_s[:, nn * 512:(nn + 1) * 512],
                        qTh[:, 2 * dp:2 * dp + 2, qt * P:(qt + 1) * P],
                        kTh[:, 2 * dp:2 * dp + 2, nn * 512:(nn + 1) * 512],
                        start=(dp == 0), stop=(dp == HT // 2 - 1), perf_mode=DR)
            s_sb = pBC.tile([P, T], F32, name="s_sb", tag="s_sb", bufs=1)
            nc.vector.scalar_tensor_tensor(s_sb[:], ps_s[:], ISQ, mb[:, qt, :],
                                           OP.mult, OP.add)
            # logits are bounded (<= ~15) so exp needs no max-subtraction
            e_sb = pBC.tile([P, T], BF16, name="e_sb", tag="e_sb", bufs=2)
            sums = pBC.tile([P, 1], F32, name="sums", tag="sums", bufs=2)
            nc.scalar.activation(out=e_sb[:], in_=s_sb[:], func=ACT.Exp,
                                 bias=0.0, scale=1.0, accum_out=sums[:])
            recip = pBC.tile([P, 1], F32, name="recip", tag="recip", bufs=2)
            nc.vector.reciprocal(recip[:], sums[:])
            nc.vector.tensor_scalar_mul(e_sb[:], e_sb[:], recip[:])
            nc.sync.dma_start_transpose(attT[:, :, qt * P:(qt + 1) * P], e_sb[:])
        if h == H - 1:  # per-qt cast so the split AV can start immediately
            for qt in range(RT):
                nc.vector.tensor_copy(attT8[:, :, qt * P:(qt + 1) * P],
                                      attT[:, :, qt * P:(qt + 1) * P])
        else:
            nc.vector.tensor_copy(attT8[:], attT[:])
        hs[h] = hs[h] + (attT8,)

    def emit_av(h):
        _, _, attT8 = hs.pop(h)
        nq = RT if h == H - 1 else 1   # last head: split over qt chunks so AV
        nw = R // nq                   # overlaps the tail softmax chain
        for dt_ in range(HT):
            ps_y = psBC.tile([P, R], F32, name="ps_y", tag="av", bufs=2)
            for qc in range(nq):
                for kp in range(TT // 2):
                    nc.tensor.matmul(
                        ps_y[:, qc * nw:(qc + 1) * nw],
                        vh[:, 2 * kp:2 * kp + 2, h * DH + dt_ * P:h * DH + (dt_ + 1) * P],
                        attT8[:, 2 * kp:2 * kp + 2, qc * nw:(qc + 1) * nw],
                        start=(kp == 0), stop=(kp == TT // 2 - 1), perf_mode=DR)
            nc.scalar.activation(out=yT[:, h * HT + dt_, :], in_=ps_y[:], func=ACT.Copy)

    emit_qk(0)
    emit_scores(0)
    # prefetch the first w1 chunks (scalar/sync queues) during attention
    w1cs = {}
    for fp in range(3):
        w1c = pW1.tile([P, CT, 2, P], BF16, name="w1c", tag="w1c", bufs=3)
        (nc.scalar if fp % 2 == 0 else nc.sync).dma_start(w1c[:], d["w1"][fp])
        w1cs[fp] = w1c
    for h in range(H):
        if h + 1 < H:
            emit_qk(h + 1)
        emit_av(h)
        if h + 1 < H:
            emit_scores(h + 1)
    psBC.release()
    pBC.release()
    p_mb.release()
    pQKw.release()
    p_vh.release()
    p_xnT.release()

    # ones1 has a single 1 in row 0: ones1.T @ bias_bc adds a bias chunk
    # inside the matmul accumulation (used in stages D and G).
    ones1 = consts.tile([P, P], BF16, name="ones1")
    nc.vector.memset(ones1[:], 0.0)
    nc.vector.memset(ones1[0:1, :], 1.0)

    # ---------------- Stage D: per-qt O-proj + residual (in SBUF) + LN2
    p_hT = tc.alloc_tile_pool(name="p_hT", bufs=1)
    hT = p_hT.tile([P, FT, R], BF16, name="hT")
    p_xn2T = tc.alloc_tile_pool(name="p_xn2T", bufs=1)
    xn2T = p_xn2T.tile([P, CT, R], BF16, name="xn2T")
    psX = tc.alloc_tile_pool(name="psX", bufs=8, space="PSUM")
    pD = tc.alloc_tile_pool(name="pD", bufs=2)
    bo_bc = _bcast_load(nc, pD, d["bo"], "bo_bc", BF16)
    for qt in range(RT):
        for fc in range(4):
            ps_o = psX.tile([P, 512], F32, name="ps_o", tag="ps", bufs=8)
            for kp in range(CP):
                nc.tensor.matmul(ps_o[:], yT[:, 2 * kp:2 * kp + 2, qt * P:(qt + 1) * P],
                                 wo_t[:, 2 * kp:2 * kp + 2, fc, :],
                                 start=(kp == 0), stop=False, perf_mode=DR)
            nc.tensor.matmul(ps_o[:], ones1[:], bo_bc[:, fc * 512:(fc + 1) * 512],
                             start=False, stop=True)
            r_sl = r_sb[:, qt, fc * 512:(fc + 1) * 512]
            nc.vector.tensor_tensor(r_sl, ps_o[:], r_sl, OP.add)
        xn2_t = _ln_tile(nc, pD, r_sb[:, qt, :], eps_t, "ln2", qt)
        nc.sync.dma_start_transpose(xn2T[:, :, qt * P:(qt + 1) * P], xn2_t[:])

    # ---------------- Stage F: MLP up + gelu -> hT [128, FT, R] bf16
    # First 3 chunks run as two FD-256 halves (gated on xn2T qt01 / qt23) so
    # the PE starts MLP-up before the last LN2 tiles finish.
    for fp in range(FT // 2):
        if fp in w1cs:
            w1c = w1cs.pop(fp)
        else:
            w1c = pW1.tile([P, CT, 2, P], BF16, name="w1c", tag="w1c", bufs=3)
            (nc.scalar if fp % 2 == 0 else nc.sync).dma_start(w1c[:], d["w1"][fp])
        for fl in range(2):
            fo = 2 * fp + fl
            ps_h = psX.tile([P, R], F32, name="ps_h", tag="ps", bufs=8)
            if fp < 3:
                for hf in range(2):
                    sl = slice(hf * 256, (hf + 1) * 256)
                    for ki in range(CT):
                        nc.tensor.matmul(ps_h[:, sl], w1c[:, ki, fl, :],
                                         xn2T[:, ki, sl],
                                         start=(ki == 0), stop=(ki == CT - 1))
            else:
                for ki in range(CT):
                    nc.tensor.matmul(ps_h[:], w1c[:, ki, fl, :], xn2T[:, ki, :],
                                     start=(ki == 0), stop=(ki == CT - 1))
            nc.scalar.activation(out=hT[:, fo, :], in_=ps_h[:], func=ACT.Gelu,
                                 bias=b1_t[:, fo:fo + 1], scale=1.0)
        if fp == 2:  # yT/wo are dead once stage D's matmuls retire
            p_yT.release()
            p_wo.release()

    # ---------------- Stage G: MLP down + residual -> out
    pG = tc.alloc_tile_pool(name="pG", bufs=2)
    b2_bc = _bcast_load(nc, pG, d["b2"], "b2_bc", BF16)
    st_engs = [nc.scalar, nc.sync, nc.gpsimd, nc.scalar]
    for fc in range(4):
        ps4 = [psX.tile([P, 512], F32, name=f"ps_g{qt}", tag="ps", bufs=8)
               for qt in range(RT)]
        for hb in range(FT // 8):
            w2b = pG.tile([P, 8, 512], BF16, name="w2b", tag="w2b", bufs=3)
            (nc.gpsimd if hb % 2 == 0 else nc.sync).dma_start(w2b[:], d["w2"][fc, hb])
            for hl in range(8):
                ho = hb * 8 + hl
                for qt in range(RT):
                    nc.tensor.matmul(ps4[qt][:], hT[:, ho, qt * P:(qt + 1) * P],
                                     w2b[:, hl, :], start=(ho == 0), stop=False)
        for qt in range(RT):
            nc.tensor.matmul(ps4[qt][:], ones1[:], b2_bc[:, fc * 512:(fc + 1) * 512],
                             start=False, stop=True)
            o_t = pG.tile([P, 512], F32, name="o_t", tag="o_t", bufs=4)
            nc.vector.tensor_tensor(o_t[:], ps4[qt][:],
                                    r_sb[:, qt, fc * 512:(fc + 1) * 512], OP.add)
            st_engs[qt].dma_start(d["out"][qt * P:(qt + 1) * P, fc * 512:(fc + 1) * 512],
                                  o_t[:])
    pG.release()
    pD.release()
    psX.release()
    p_xn2T.release()
    p_hT.release()
    pW1.release()
    p_r.release()
    consts.release()


def build_program():
    nc = bacc.Bacc("TRN2", target_bir_lowering=False, debug=False, num_devices=8)
    with tile.TileContext(nc) as tc:
        _body(tc)
    nc.compile()
    return nc


_prog = None


def _get_prog():
    global _prog
    if _prog is None:
        _prog = build_program()
    return _prog


def make_in_maps(x, mask, Wq, bq, Wk, bk, Wv, bv, Wo, bo,
                 ln1_w, ln1_b, ln2_w, ln2_b, W1, b1, W2, b2):
    bf = ml_dtypes.bfloat16
    f8 = ml_dtypes.float8_e4m3
    f32 = np.float32
    cc = np.ascontiguousarray

    def f(a):
        return np.asarray(a, dtype=f32)

    x, mask = np.asarray(x, dtype=f32), np.asarray(mask)
    # fold the LN affines into the consuming matmuls: for y = ln(x)@W + b with
    # ln(x) = z*w + b_ln (z the normalized input), y = z@(w[:,None]*W) + (b_ln@W + b)
    w1l, b1l = f(ln1_w)[:, None], f(ln1_b)
    w2l, b2l = f(ln2_w)[:, None], f(ln2_b)
    Wq_, Wk_, Wv_, W1_ = w1l * f(Wq), w1l * f(Wk), w1l * f(Wv), w2l * f(W1)
    bq_, bk_ = f(bq) + b1l @ f(Wq), f(bk) + b1l @ f(Wk)
    bv_, b1_ = f(bv) + b1l @ f(Wv), f(b1) + b2l @ f(W1)
    wq_h = cc(Wq_.astype(f8).reshape(CT, P, CT, P).transpose(2, 1, 0, 3))
    wk_h = cc(Wk_.astype(f8).reshape(CT, P, CT, P).transpose(2, 1, 0, 3))
    wv_h = cc(Wv_.astype(f8).reshape(CT, P, C).transpose(1, 0, 2))
    wo_h = cc(f(Wo).astype(f8).reshape(CT, P, 4, 512).transpose(1, 0, 2, 3))
    w1_h = cc(W1_.astype(bf).reshape(CT, P, FT // 2, 2, P).transpose(2, 1, 0, 3, 4))
    w2_h = cc(f(W2).astype(bf).reshape(FT // 8, 8, P, 4, 512).transpose(3, 0, 2, 1, 4))
    shared = dict(
        wq=wq_h, wk=wk_h, wv=wv_h, wo=wo_h, w1=w1_h, w2=w2_h,
        bq=cc(bq_.reshape(CT, P).T), bk=cc(bk_.reshape(CT, P).T),
        b1=cc(b1_.reshape(FT, P).T),
        bv=bv_.astype(bf), bo=f(bo).astype(bf), b2=f(b2).astype(bf),
    )
    in_maps = []
    for c in range(8):
        b, hh = divmod(c, 2)
        xc = np.roll(x[b], -hh * R, axis=0)
        mk = np.roll(np.asarray(mask[b, hh * R:(hh + 1) * R, :], dtype=f32),
                     -hh * R, axis=1).astype(bf)
        in_maps.append({**shared, "xb": cc(xc.astype(bf)), "xq": cc(xc[:R]),
                        "mask": cc(mk)})
    return in_maps


def kernel(**inputs):
    nc = _get_prog()
    in_maps = make_in_maps(**inputs)
    res = run_bass_kernel_spmd(nc, in_maps, core_ids=list(range(8)))
    out = np.empty((B, T, C), np.float32)
    for c in range(8):
        b, hh = divmod(c, 2)
        out[b, hh * R:(hh + 1) * R, :] = res.results[c]["out"]
    return out


# revision 9
# speedup vs baseline: 1.0031x; 1.0031x over previous
"""Transformer block (LN -> MHA -> residual -> LN -> MLP -> residual) on 8 TRN2
NeuronCores.

Sharding: pure row data-parallelism over (batch, sequence-half). Core c handles
batch b = c//2 and query rows [h*512, (h+1)*512) with h = c%2. Each core
computes K/V projections for its full batch locally (small duplicated work),
which removes every cross-core collective. Host reorders each core's batch rows
"own rows first" so the same SPMD program works on all cores; mask columns are
permuted identically (softmax/attention are permutation-invariant over keys).

v3: DMA/queue rebalance + bubble removal on top of v2's fp8 attention.
  - x tiles stream on 4 DMA queues (sync/vector/gpsimd/tensor) instead of one,
    so LN1 is fed 4x faster and stage A becomes PE-bound almost immediately.
  - all Q/K weights issue up front on the gpsimd queue; pool-slot rotation
    (bufs=4 per tag) paces them one head ahead of compute automatically.
  - wo preloads on the tensor queue behind the x tiles; w1 streams on
    scalar+sync, w2 on gpsimd+vector; output stores rotate over queues.
  - residual r stays fully in SBUF (no DRAM bounce); the f32 x rows load into
    the same tile during attention and the O-proj adds in place.
  - MLP-up's first 3 weight chunks run as two FD-256 halves (gated on qt01 /
    qt23) so the PE starts the MLP while the last LN2 tiles are finishing.
  - gelu activation table preloaded during stage A (off the D->F critical
    path); stage-D xn2T transposes on the otherwise-idle gpsimd engine.
fp32 PSUM accumulation everywhere; statistics stay fp32.
"""

import numpy as np
import ml_dtypes

import concourse.bass as bass
import concourse.tile as tile
from concourse import bacc, mybir
from concourse.bass_utils import run_bass_kernel_spmd

BF16 = mybir.dt.bfloat16
F32 = mybir.dt.float32
FP8 = mybir.dt.float8e4
AX = mybir.AxisListType
OP = mybir.AluOpType
ACT = mybir.ActivationFunctionType
DR = mybir.MatmulPerfMode.DoubleRow

P = 128
B, T, C, H = 4, 1024, 2048, 4
DH = C // H                      # 512
F = 4 * C                        # 8192
R = T // 2                       # 512 own query rows per core
RT, TT, CT, FT = R // P, T // P, C // P, F // P   # 4, 8, 16, 64
CP = CT // 2                     # 8 double-row contraction steps over C
HT = DH // P                     # 4 feature tiles per head
EPS = 1e-5
ISQ = 1.0 / float(np.sqrt(DH))
NEGBIG = 30000.0


def _bcast_load(nc, pool, dram_ap, name, dtype):
    """Broadcast a [n] DRAM vector to all 128 partitions -> [128, n]."""
    t = pool.tile([P, dram_ap.shape[0]], dtype, name=name)
    src = bass.AP(
        tensor=dram_ap.tensor, offset=dram_ap.offset, ap=[[0, P]] + list(dram_ap.ap)
    )
    nc.gpsimd.dma_start(out=t[:], in_=src)
    return t


def _ln_tile(nc, pool, x_sl, eps_t, tag, i, xh_bufs=1):
    """Normalize one [128, C] tile -> bf16 (x-mu)*rstd. The LN affine (w,b)
    is folded into the following matmul's weights/biases on the host."""
    stats = pool.tile([P, 4, 6], F32, name=f"{tag}_stats{i}", tag=f"{tag}_stats",
                      bufs=2)
    for sg in range(4):
        nc.vector.bn_stats(out=stats[:, sg, :], in_=x_sl[:, sg * 512:(sg + 1) * 512])
    mv = pool.tile([P, 2], F32, name=f"{tag}_mv{i}", tag=f"{tag}_mv", bufs=2)
    nc.vector.bn_aggr(out=mv[:], in_=stats[:])
    std = pool.tile([P, 1], F32, name=f"{tag}_std{i}", tag=f"{tag}_std", bufs=2)
    nc.scalar.activation(out=std[:], in_=mv[:, 1:2], func=ACT.Sqrt,
                         bias=eps_t[:], scale=1.0)
    rstd = pool.tile([P, 1], F32, name=f"{tag}_rstd{i}", tag=f"{tag}_rstd", bufs=2)
    nc.vector.reciprocal(rstd[:], std[:])
    nmr = pool.tile([P, 1], F32, name=f"{tag}_nmr{i}", tag=f"{tag}_nmr", bufs=2)
    nc.vector.tensor_scalar(nmr[:], mv[:, 0:1], rstd[:], -1.0, OP.mult, OP.mult)
    xh = pool.tile([P, C], BF16, name=f"{tag}_xh{i}", tag=f"{tag}_xh", bufs=xh_bufs)
    nc.scalar.activation(out=xh[:], in_=x_sl, func=ACT.Identity,
                         bias=nmr[:], scale=rstd[:])
    return xh


def _body(tc):
    nc = tc.nc
    d = {n: nc.dram_tensor(n, s, dt, kind=k).ap() for n, s, dt, k in [
        ("xb", [T, C], BF16, "ExternalInput"),
        ("xq", [R, C], F32, "ExternalInput"),
        ("mask", [R, T], FP8, "ExternalInput"),
        ("wq", [CT, P, CT, P], FP8, "ExternalInput"),
        ("wk", [CT, P, CT, P], FP8, "ExternalInput"),
        ("wv", [P, CT, C], FP8, "ExternalInput"),
        ("wo", [P, CT, 4, 512], FP8, "ExternalInput"),
        ("w1", [FT // 2, P, CT, 2, P], BF16, "ExternalInput"),
        ("w2", [4, FT // 8, P, 8, 512], BF16, "ExternalInput"),
        ("bq", [P, CT], F32, "ExternalInput"),
        ("bk", [P, CT], F32, "ExternalInput"),
        ("b1", [P, FT], F32, "ExternalInput"),
        ("bv", [C], BF16, "ExternalInput"),
        ("b2", [C], BF16, "ExternalInput"),
        ("out", [R, C], F32, "ExternalOutput"),
    ]}

    consts = tc.alloc_tile_pool(name="consts", bufs=1)
    eps_t = consts.tile([P, 1], F32, name="eps")
    nc.vector.memset(eps_t[:], EPS)
    # r rows live in SBUF for the whole kernel: loaded with the f32 x rows
    # during attention, O-proj residual adds in place, LN2 + final adds read it.
    p_r = tc.alloc_tile_pool(name="p_r", bufs=1)
    r_sb = p_r.tile([P, RT, C], F32, name="r_sb")
    # right stack: the w1 stream pool at the bottom (lives into stage G);
    # yT/wo above it release once stage D's matmuls retire.
    pW1 = tc.alloc_tile_pool(name="pW1", bufs=1, side="right")
    p_wo = tc.alloc_tile_pool(name="p_wo", bufs=1, side="right")
    wo_t = p_wo.tile([P, CT, 4, 512], FP8, name="wo_t")
    p_yT = tc.alloc_tile_pool(name="p_yT", bufs=1, side="right")
    yT = p_yT.tile([P, CT, R], FP8, name="yT")
    # attention-era pools (all released after AV of the last head)
    p_xnT = tc.alloc_tile_pool(name="p_xnT", bufs=1)
    xnT8_lo = p_xnT.tile([P, CT, R], FP8, name="xnT8_lo")
    xnT8_hi = p_xnT.tile([P, CT, R], FP8, name="xnT8_hi")
    xnT8 = [xnT8_lo, xnT8_hi]
    p_vh = tc.alloc_tile_pool(name="p_vh", bufs=1)
    vh = p_vh.tile([P, TT, C], FP8, name="vh")
    pQKw = tc.alloc_tile_pool(name="pQKw", bufs=1)
    p_mb = tc.alloc_tile_pool(name="p_mb", bufs=1)
    mb = p_mb.tile([P, RT, T], FP8, name="mb")

    # ---------------- Stage A: per-tile LN1 -> transpose -> fp8 cast -> V proj
    p_wv = tc.alloc_tile_pool(name="p_wv", bufs=1)
    wv_t = p_wv.tile([P, CT, C], FP8, name="wv_t")
    lnA = tc.alloc_tile_pool(name="lnA", bufs=1)
    bv_bc = _bcast_load(nc, lnA, d["bv"], "bv_bc", BF16)
    pA = tc.alloc_tile_pool(name="pA", bufs=2)
    psA = tc.alloc_tile_pool(name="psA", bufs=2, space="PSUM")

    # Ring discipline: the sync ring carries ONLY the latency-critical x tiles
    # (and later the transposes + final stores). All bulk streams on gpsimd in
    # explicit priority order; pool-slot rotation paces the Q/K weight stream
    # one head ahead of compute. The scalar ring stays clear for the stage-A
    # transposes, then carries half the w1/w2 streams.
    xts = []
    for tt in range(TT):
        xt = pA.tile([P, C], BF16, name=f"xt{tt}", tag="xt", bufs=2)
        nc.sync.dma_start(xt[:], d["xb"][tt * P:(tt + 1) * P, :])
        xts.append(xt[:])
    for kc in range(4):
        nc.gpsimd.dma_start(wv_t[:, 4 * kc:4 * (kc + 1), :],
                            d["wv"][:, 4 * kc:4 * (kc + 1), :])
    wqcs, wkcs = {}, {}

    def qk_dma(h):
        for fl in range(HT):
            fo = h * HT + fl
            wqc = pQKw.tile([P, CT, P], FP8, name=f"wqc{fo}", tag="wqc", bufs=4)
            nc.gpsimd.dma_start(wqc[:], d["wq"][fo])
            wqcs[fo] = wqc
            wkc = pQKw.tile([P, CT, P], FP8, name=f"wkc{fo}", tag="wkc", bufs=4)
            nc.gpsimd.dma_start(wkc[:], d["wk"][fo])
            wkcs[fo] = wkc

    qk_dma(0)
    nc.gpsimd.dma_start(out=mb[:], in_=d["mask"].rearrange("(qo qp) k -> qp qo k", qp=P))
    bq_t = consts.tile([P, CT], F32, name="bq_t")
    nc.gpsimd.dma_start(out=bq_t[:], in_=d["bq"])
    bk_t = consts.tile([P, CT], F32, name="bk_t")
    nc.gpsimd.dma_start(out=bk_t[:], in_=d["bk"])
    b1_t = consts.tile([P, FT], F32, name="b1_t")
    nc.gpsimd.dma_start(out=b1_t[:], in_=d["b1"])
    for h in range(1, H):
        qk_dma(h)
    # bulk preloads sit behind the (self-pacing) qk stream: wo for stage D,
    # then the f32 residual rows (bo already folded in on the host).
    for kc in range(4):
        nc.gpsimd.dma_start(wo_t[:, 4 * kc:4 * (kc + 1), :, :],
                            d["wo"][:, 4 * kc:4 * (kc + 1), :, :])
    nc.gpsimd.dma_start(out=r_sb[:], in_=d["xq"].rearrange("(qo qp) c -> qp qo c", qp=P))
    # preload the gelu activation table off the D->F critical path
    gdum = consts.tile([P, 1], F32, name="gdum")
    nc.scalar.activation(out=gdum[:], in_=eps_t[:], func=ACT.Gelu,
                         bias=0.0, scale=1.0)

    # software-pipelined by one tile: cast(tt)+V(tt) are emitted during
    # LN(tt+1) so the fp8 cast's transpose-wait never blocks the next LN
    # apply in the scalar FIFO.
    xnTts = {}

    def _emit_castv(tt):
        half, lt = divmod(tt, 4)
        nc.vector.tensor_copy(xnT8[half][:, :, lt * P:(lt + 1) * P],
                              xnTts.pop(tt)[:])
        for h in range(H):
            ps_v = psA.tile([P, DH], F32, name="ps_v", tag="psA", bufs=2)
            for kp in range(CP):
                nc.tensor.matmul(ps_v[:],
                                 xnT8[half][:, 2 * kp:2 * kp + 2, lt * P:(lt + 1) * P],
                                 wv_t[:, 2 * kp:2 * kp + 2, h * DH:(h + 1) * DH],
                                 start=(kp == 0), stop=(kp == CP - 1), perf_mode=DR)
            nc.vector.tensor_tensor(vh[:, tt, h * DH:(h + 1) * DH], ps_v[:],
                                    bv_bc[:, h * DH:(h + 1) * DH], OP.add)

    for tt in range(TT):
        xn_t = _ln_tile(nc, pA, xts[tt], eps_t, "ln1", tt)
        xnTt = pA.tile([P, CT, P], BF16, name=f"xnTt{tt}", tag="xnTt", bufs=2)
        nc.scalar.dma_start_transpose(xnTt[:], xn_t[:])
        xnTts[tt] = xnTt
        if tt > 0:
            _emit_castv(tt - 1)
    _emit_castv(TT - 1)
    psA.release()
    pA.release()
    lnA.release()
    p_wv.release()

    # ---------------- Stage B+C: software-pipelined per-head Q/K + attention
    pBC = tc.alloc_tile_pool(name="pBC", bufs=2)
    psBC = tc.alloc_tile_pool(name="psBC", bufs=2, space="PSUM")
    hs = {}

    def emit_qk(h):
        qTh = pBC.tile([P, HT, R], FP8, name=f"qTh{h}", tag="qTh", bufs=2)
        kTh = pBC.tile([P, HT, T], FP8, name=f"kTh{h}", tag="kTh", bufs=2)
        for fl in range(HT):
            fo = h * HT + fl
            wqc, wkc = wqcs.pop(fo), wkcs.pop(fo)
            ps_q = psBC.tile([P, R], F32, name="ps_q", tag="psB", bufs=2)
            for kp in range(CP):
                nc.tensor.matmul(ps_q[:], wqc[:, 2 * kp:2 * kp + 2, :],
                                 xnT8_lo[:, 2 * kp:2 * kp + 2, :],
                                 start=(kp == 0), stop=(kp == CP - 1), perf_mode=DR)
            nc.scalar.activation(out=qTh[:, fl, :], in_=ps_q[:], func=ACT.Identity,
                                 bias=bq_t[:, fo:fo + 1], scale=1.0)
            for nn in range(2):
                ps_k = psBC.tile([P, 512], F32, name="ps_k", tag="psB", bufs=2)
                for kp in range(CP):
                    nc.tensor.matmul(ps_k[:], wkc[:, 2 * kp:2 * kp + 2, :],
                                     xnT8[nn][:, 2 * kp:2 * kp + 2, :],
                                     start=(kp == 0), stop=(kp == CP - 1), perf_mode=DR)
                nc.scalar.activation(out=kTh[:, fl, nn * 512:(nn + 1) * 512], in_=ps_k[:],
                                     func=ACT.Identity, bias=bk_t[:, fo:fo + 1],
                                     scale=1.0)
        hs[h] = (qTh, kTh)

    def emit_scores(h):
        qTh, kTh = hs[h]
        attT = pBC.tile([P, TT, R], BF16, name=f"attT{h}", tag="attT", bufs=1)
        attT8 = pBC.tile([P, TT, R], FP8, name=f"attT8{h}", tag="attT8", bufs=2)
        for qt in range(RT):
            ps_s = psBC.tile([P, T], F32, name="ps_s", tag="scores", bufs=2)
            for nn in range(2):
                for dp in range(HT // 2):
                    nc.tensor.matmul(
                        ps_s[:, nn * 512:(nn + 1) * 512],
                        qTh[:, 2 * dp:2 * dp + 2, qt * P:(qt + 1) * P],
                        kTh[:, 2 * dp:2 * dp + 2, nn * 512:(nn + 1) * 512],
                        start=(dp == 0), stop=(dp == HT // 2 - 1), perf_mode=DR)
            s_sb = pBC.tile([P, T], F32, name="s_sb", tag="s_sb", bufs=1)
            nc.vector.scalar_tensor_tensor(s_sb[:], ps_s[:], ISQ, mb[:, qt, :],
                                           OP.mult, OP.add)
            # logits are bounded (<= ~15) so exp needs no max-subtraction
            e_sb = pBC.tile([P, T], BF16, name="e_sb", tag="e_sb", bufs=2)
            sums = pBC.tile([P, 1], F32, name="sums", tag="sums", bufs=2)
            nc.scalar.activation(out=e_sb[:], in_=s_sb[:], func=ACT.Exp,
                                 bias=0.0, scale=1.0, accum_out=sums[:])
            recip = pBC.tile([P, 1], F32, name="recip", tag="recip", bufs=2)
            nc.vector.reciprocal(recip[:], sums[:])
            nc.vector.tensor_scalar_mul(e_sb[:], e_sb[:], recip[:])
            nc.sync.dma_start_transpose(attT[:, :, qt * P:(qt + 1) * P], e_sb[:])
        if h == H - 1:  # per-qt cast so the split AV can start immediately
            for qt in range(RT):
                nc.vector.tensor_copy(attT8[:, :, qt * P:(qt + 1) * P],
                                      attT[:, :, qt * P:(qt + 1) * P])
        else:
            nc.vector.tensor_copy(attT8[:], attT[:])
        hs[h] = hs[h] + (attT8,)

    def emit_av(h):
        _, _, attT8 = hs.pop(h)
        nq = RT if h == H - 1 else 1   # last head: split over qt chunks so AV
        nw = R // nq                   # overlaps the tail softmax chain
        for dt_ in range(HT):
            ps_y = psBC.tile([P, R], F32, name="ps_y", tag="av", bufs=2)
            for qc in range(nq):
                for kp in range(TT // 2):
                    nc.tensor.matmul(
                        ps_y[:, qc * nw:(qc + 1) * nw],
                        vh[:, 2 * kp:2 * kp + 2, h * DH + dt_ * P:h * DH + (dt_ + 1) * P],
                        attT8[:, 2 * kp:2 * kp + 2, qc * nw:(qc + 1) * nw],
                        start=(kp == 0), stop=(kp == TT // 2 - 1), perf_mode=DR)
            nc.scalar.activation(out=yT[:, h * HT + dt_, :], in_=ps_y[:], func=ACT.Copy)

    emit_qk(0)
    emit_scores(0)
    # prefetch the first w1 chunks (scalar/sync queues) during attention
    w1cs = {}
    for fp in range(3):
        w1c = pW1.tile([P, CT, 2, P], BF16, name="w1c", tag="w1c", bufs=3)
        (nc.scalar if fp % 2 == 0 else nc.gpsimd).dma_start(w1c[:], d["w1"][fp])
        w1cs[fp] = w1c
    for h in range(H):
        if h + 1 < H:
            emit_qk(h + 1)
        emit_av(h)
        if h + 1 < H:
            emit_scores(h + 1)
    psBC.release()
    pBC.release()
    p_mb.release()
    pQKw.release()
    p_vh.release()
    p_xnT.release()

    # ones1 has a single 1 in row 0: ones1.T @ bias_bc adds a bias chunk
    # inside the matmul accumulation (used in stages D and G).
    ones1 = consts.tile([P, P], BF16, name="ones1")
    nc.vector.memset(ones1[:], 0.0)
    nc.vector.memset(ones1[0:1, :], 1.0)

    # ---------------- Stage D: per-qt O-proj + residual (in SBUF) + LN2
    p_hT = tc.alloc_tile_pool(name="p_hT", bufs=1)
    hT = p_hT.tile([P, FT, R], BF16, name="hT")
    p_xn2T = tc.alloc_tile_pool(name="p_xn2T", bufs=1)
    xn2T = p_xn2T.tile([P, CT, R], BF16, name="xn2T")
    psX = tc.alloc_tile_pool(name="psX", bufs=8, space="PSUM")
    pD = tc.alloc_tile_pool(name="pD", bufs=2)
    for qt in range(RT):
        for fc in range(4):
            ps_o = psX.tile([P, 512], F32, name="ps_o", tag="ps", bufs=8)
            for kp in range(CP):
                nc.tensor.matmul(ps_o[:], yT[:, 2 * kp:2 * kp + 2, qt * P:(qt + 1) * P],
                                 wo_t[:, 2 * kp:2 * kp + 2, fc, :],
                                 start=(kp == 0), stop=(kp == CP - 1), perf_mode=DR)
            r_sl = r_sb[:, qt, fc * 512:(fc + 1) * 512]
            nc.vector.tensor_tensor(r_sl, ps_o[:], r_sl, OP.add)
        xn2_t = _ln_tile(nc, pD, r_sb[:, qt, :], eps_t, "ln2", qt, xh_bufs=2)
        nc.sync.dma_start_transpose(xn2T[:, :, qt * P:(qt + 1) * P], xn2_t[:])

    # ---------------- Stage F: MLP up + gelu -> hT [128, FT, R] bf16
    # First 3 chunks run as two FD-256 halves (gated on xn2T qt01 / qt23) so
    # the PE starts MLP-up before the last LN2 tiles finish.
    for fp in range(FT // 2):
        if fp in w1cs:
            w1c = w1cs.pop(fp)
        else:
            w1c = pW1.tile([P, CT, 2, P], BF16, name="w1c", tag="w1c", bufs=3)
            (nc.scalar if fp % 2 == 0 else nc.gpsimd).dma_start(w1c[:], d["w1"][fp])
        for fl in range(2):
            fo = 2 * fp + fl
            ps_h = psX.tile([P, R], F32, name="ps_h", tag="ps", bufs=8)
            if fp < 3:
                for hf in range(2):
                    sl = slice(hf * 256, (hf + 1) * 256)
                    for ki in range(CT):
                        nc.tensor.matmul(ps_h[:, sl], w1c[:, ki, fl, :],
                                         xn2T[:, ki, sl],
                                         start=(ki == 0), stop=(ki == CT - 1))
            else:
                for ki in range(CT):
                    nc.tensor.matmul(ps_h[:], w1c[:, ki, fl, :], xn2T[:, ki, :],
                                     start=(ki == 0), stop=(ki == CT - 1))
            nc.scalar.activation(out=hT[:, fo, :], in_=ps_h[:], func=ACT.Gelu,
                                 bias=b1_t[:, fo:fo + 1], scale=1.0)
        if fp == 2:  # yT/wo are dead once stage D's matmuls retire
            p_yT.release()
            p_wo.release()

    # ---------------- Stage G: MLP down + residual -> out
    pG = tc.alloc_tile_pool(name="pG", bufs=2)
    b2_bc = _bcast_load(nc, pG, d["b2"], "b2_bc", BF16)
    for fc in range(4):
        ps4 = [psX.tile([P, 512], F32, name=f"ps_g{qt}", tag="ps", bufs=8)
               for qt in range(RT)]
        for hb in range(FT // 8):
            w2b = pG.tile([P, 8, 512], BF16, name="w2b", tag="w2b", bufs=3)
            (nc.gpsimd if hb % 2 == 0 else nc.scalar).dma_start(w2b[:], d["w2"][fc, hb])
            for hl in range(8):
                ho = hb * 8 + hl
                for qt in range(RT):
                    nc.tensor.matmul(ps4[qt][:], hT[:, ho, qt * P:(qt + 1) * P],
                                     w2b[:, hl, :], start=(ho == 0), stop=False)
        for qt in range(RT):
            nc.tensor.matmul(ps4[qt][:], ones1[:], b2_bc[:, fc * 512:(fc + 1) * 512],
                             start=False, stop=True)
            o_t = pG.tile([P, 512], F32, name="o_t", tag="o_t", bufs=3)
            nc.vector.tensor_tensor(o_t[:], ps4[qt][:],
                                    r_sb[:, qt, fc * 512:(fc + 1) * 512], OP.add)
            nc.sync.dma_start(d["out"][qt * P:(qt + 1) * P, fc * 512:(fc + 1) * 512],
                                  o_t[:])
    pG.release()
    pD.release()
    psX.release()
    p_xn2T.release()
    p_hT.release()
    pW1.release()
    p_r.release()
    consts.release()


def build_program():
    nc = bacc.Bacc("TRN2", target_bir_lowering=False, debug=False, num_devices=8)
    with tile.TileContext(nc) as tc:
        _body(tc)
    nc.compile()
    return nc


_prog = None


def _get_prog():
    global _prog
    if _prog is None:
        _prog = build_program()
    return _prog


def make_in_maps(x, mask, Wq, bq, Wk, bk, Wv, bv, Wo, bo,
                 ln1_w, ln1_b, ln2_w, ln2_b, W1, b1, W2, b2):
    bf = ml_dtypes.bfloat16
    f8 = ml_dtypes.float8_e4m3
    f32 = np.float32
    cc = np.ascontiguousarray

    def f(a):
        return np.asarray(a, dtype=f32)

    x, mask = np.asarray(x, dtype=f32), np.asarray(mask)
    # fold the LN affines into the consuming matmuls: for y = ln(x)@W + b with
    # ln(x) = z*w + b_ln (z the normalized input), y = z@(w[:,None]*W) + (b_ln@W + b)
    w1l, b1l = f(ln1_w)[:, None], f(ln1_b)
    w2l, b2l = f(ln2_w)[:, None], f(ln2_b)
    Wq_, Wk_, Wv_, W1_ = w1l * f(Wq), w1l * f(Wk), w1l * f(Wv), w2l * f(W1)
    bq_, bk_ = f(bq) + b1l @ f(Wq), f(bk) + b1l @ f(Wk)
    bv_, b1_ = f(bv) + b1l @ f(Wv), f(b1) + b2l @ f(W1)
    wq_h = cc(Wq_.astype(f8).reshape(CT, P, CT, P).transpose(2, 1, 0, 3))
    wk_h = cc(Wk_.astype(f8).reshape(CT, P, CT, P).transpose(2, 1, 0, 3))
    wv_h = cc(Wv_.astype(f8).reshape(CT, P, C).transpose(1, 0, 2))
    wo_h = cc(f(Wo).astype(f8).reshape(CT, P, 4, 512).transpose(1, 0, 2, 3))
    w1_h = cc(W1_.astype(bf).reshape(CT, P, FT // 2, 2, P).transpose(2, 1, 0, 3, 4))
    w2_h = cc(f(W2).astype(bf).reshape(FT // 8, 8, P, 4, 512).transpose(3, 0, 2, 1, 4))
    shared = dict(
        wq=wq_h, wk=wk_h, wv=wv_h, wo=wo_h, w1=w1_h, w2=w2_h,
        bq=cc(bq_.reshape(CT, P).T), bk=cc(bk_.reshape(CT, P).T),
        b1=cc(b1_.reshape(FT, P).T),
        bv=bv_.astype(bf), b2=f(b2).astype(bf),
    )
    in_maps = []
    bo32 = f(bo)
    for c in range(8):
        b, hh = divmod(c, 2)
        xc = np.roll(x[b], -hh * R, axis=0)
        mk = np.roll((np.asarray(mask[b, hh * R:(hh + 1) * R, :], dtype=f32) - 1.0)
                     * 240.0, -hh * R, axis=1).astype(f8)
        in_maps.append({**shared, "xb": cc(xc.astype(bf)),
                        "xq": cc(xc[:R] + bo32), "mask": cc(mk)})
    return in_maps


def kernel(**inputs):
    nc = _get_prog()
    in_maps = make_in_maps(**inputs)
    res = run_bass_kernel_spmd(nc, in_maps, core_ids=list(range(8)))
    out = np.empty((B, T, C), np.float32)
    for c in range(8):
        b, hh = divmod(c, 2)
        out[b, hh * R:(hh + 1) * R, :] = res.results[c]["out"]
    return out


# revision 11
# speedup vs baseline: 1.0080x; 1.0048x over previous
"""Transformer block (LN -> MHA -> residual -> LN -> MLP -> residual) on 8 TRN2
NeuronCores.

Sharding: pure row data-parallelism over (batch, sequence-half). Core c handles
batch b = c//2 and query rows [h*512, (h+1)*512) with h = c%2. Each core
computes K/V projections for its full batch locally (small duplicated work),
which removes every cross-core collective. Host reorders each core's batch rows
"own rows first" so the same SPMD program works on all cores; mask columns are
permuted identically (softmax/attention are permutation-invariant over keys).

v3: DMA/queue rebalance + bubble removal on top of v2's fp8 attention.
  - x tiles stream on 4 DMA queues (sync/vector/gpsimd/tensor) instead of one,
    so LN1 is fed 4x faster and stage A becomes PE-bound almost immediately.
  - all Q/K weights issue up front on the gpsimd queue; pool-slot rotation
    (bufs=4 per tag) paces them one head ahead of compute automatically.
  - wo preloads on the tensor queue behind the x tiles; w1 streams on
    scalar+sync, w2 on gpsimd+vector; output stores rotate over queues.
  - residual r stays fully in SBUF (no DRAM bounce); the f32 x rows load into
    the same tile during attention and the O-proj adds in place.
  - MLP-up's first 3 weight chunks run as two FD-256 halves (gated on qt01 /
    qt23) so the PE starts the MLP while the last LN2 tiles are finishing.
  - gelu activation table preloaded during stage A (off the D->F critical
    path); stage-D xn2T transposes on the otherwise-idle gpsimd engine.
fp32 PSUM accumulation everywhere; statistics stay fp32.
"""

import numpy as np
import ml_dtypes

import concourse.bass as bass
import concourse.tile as tile
from concourse import bacc, mybir
from concourse.bass_utils import run_bass_kernel_spmd

BF16 = mybir.dt.bfloat16
F32 = mybir.dt.float32
FP8 = mybir.dt.float8e4
AX = mybir.AxisListType
OP = mybir.AluOpType
ACT = mybir.ActivationFunctionType
DR = mybir.MatmulPerfMode.DoubleRow

P = 128
B, T, C, H = 4, 1024, 2048, 4
DH = C // H                      # 512
F = 4 * C                        # 8192
R = T // 2                       # 512 own query rows per core
RT, TT, CT, FT = R // P, T // P, C // P, F // P   # 4, 8, 16, 64
CP = CT // 2                     # 8 double-row contraction steps over C
HT = DH // P                     # 4 feature tiles per head
EPS = 1e-5
ISQ = 1.0 / float(np.sqrt(DH))
NEGBIG = 30000.0


def _bcast_load(nc, pool, dram_ap, name, dtype):
    """Broadcast a [n] DRAM vector to all 128 partitions -> [128, n]."""
    t = pool.tile([P, dram_ap.shape[0]], dtype, name=name)
    src = bass.AP(
        tensor=dram_ap.tensor, offset=dram_ap.offset, ap=[[0, P]] + list(dram_ap.ap)
    )
    nc.gpsimd.dma_start(out=t[:], in_=src)
    return t


def _ln_tile(nc, pool, x_sl, eps_t, tag, i, xh_bufs=1):
    """Normalize one [128, C] tile -> bf16 (x-mu)*rstd. The LN affine (w,b)
    is folded into the following matmul's weights/biases on the host."""
    stats = pool.tile([P, 4, 6], F32, name=f"{tag}_stats{i}", tag=f"{tag}_stats",
                      bufs=2)
    for sg in range(4):
        nc.vector.bn_stats(out=stats[:, sg, :], in_=x_sl[:, sg * 512:(sg + 1) * 512])
    mv = pool.tile([P, 2], F32, name=f"{tag}_mv{i}", tag=f"{tag}_mv", bufs=2)
    nc.vector.bn_aggr(out=mv[:], in_=stats[:])
    std = pool.tile([P, 1], F32, name=f"{tag}_std{i}", tag=f"{tag}_std", bufs=2)
    nc.scalar.activation(out=std[:], in_=mv[:, 1:2], func=ACT.Sqrt,
                         bias=eps_t[:], scale=1.0)
    rstd = pool.tile([P, 1], F32, name=f"{tag}_rstd{i}", tag=f"{tag}_rstd", bufs=2)
    nc.vector.reciprocal(rstd[:], std[:])
    nmr = pool.tile([P, 1], F32, name=f"{tag}_nmr{i}", tag=f"{tag}_nmr", bufs=2)
    nc.vector.tensor_scalar(nmr[:], mv[:, 0:1], rstd[:], -1.0, OP.mult, OP.mult)
    xh = pool.tile([P, C], BF16, name=f"{tag}_xh{i}", tag=f"{tag}_xh", bufs=xh_bufs)
    nc.scalar.activation(out=xh[:], in_=x_sl, func=ACT.Identity,
                         bias=nmr[:], scale=rstd[:])
    return xh


def _body(tc):
    nc = tc.nc
    d = {n: nc.dram_tensor(n, s, dt, kind=k).ap() for n, s, dt, k in [
        ("xb", [T, C], BF16, "ExternalInput"),
        ("xq", [R, C], F32, "ExternalInput"),
        ("mask", [R, T], FP8, "ExternalInput"),
        ("wq", [CT, P, CT, P], FP8, "ExternalInput"),
        ("wk", [CT, P, CT, P], FP8, "ExternalInput"),
        ("wv", [P, CT, C], FP8, "ExternalInput"),
        ("wo", [P, CT, 4, 512], FP8, "ExternalInput"),
        ("w1", [FT // 2, P, CT, 2, P], BF16, "ExternalInput"),
        ("w2", [4, FT // 8, P, 8, 512], BF16, "ExternalInput"),
        ("bq", [P, CT], F32, "ExternalInput"),
        ("bk", [P, CT], F32, "ExternalInput"),
        ("b1", [P, FT], F32, "ExternalInput"),
        ("bv", [C], BF16, "ExternalInput"),
        ("b2", [C], BF16, "ExternalInput"),
        ("out", [R, C], F32, "ExternalOutput"),
    ]}

    consts = tc.alloc_tile_pool(name="consts", bufs=1)
    eps_t = consts.tile([P, 1], F32, name="eps")
    nc.vector.memset(eps_t[:], EPS)
    # r rows live in SBUF for the whole kernel: loaded with the f32 x rows
    # during attention, O-proj residual adds in place, LN2 + final adds read it.
    p_r = tc.alloc_tile_pool(name="p_r", bufs=1)
    r_sb = p_r.tile([P, RT, C], F32, name="r_sb")
    # right stack: the w1 stream pool at the bottom (lives into stage G);
    # yT/wo above it release once stage D's matmuls retire.
    pW1 = tc.alloc_tile_pool(name="pW1", bufs=1, side="right")
    p_wo = tc.alloc_tile_pool(name="p_wo", bufs=1, side="right")
    wo_t = p_wo.tile([P, CT, 4, 512], FP8, name="wo_t")
    p_yT = tc.alloc_tile_pool(name="p_yT", bufs=1, side="right")
    yT = p_yT.tile([P, CT, R], FP8, name="yT")
    # attention-era pools (all released after AV of the last head)
    p_xnT = tc.alloc_tile_pool(name="p_xnT", bufs=1)
    xnT8_lo = p_xnT.tile([P, CT, R], FP8, name="xnT8_lo")
    xnT8_hi = p_xnT.tile([P, CT, R], FP8, name="xnT8_hi")
    xnT8 = [xnT8_lo, xnT8_hi]
    p_vh = tc.alloc_tile_pool(name="p_vh", bufs=1)
    vh = p_vh.tile([P, TT, C], FP8, name="vh")
    pQKw = tc.alloc_tile_pool(name="pQKw", bufs=1)
    p_mb = tc.alloc_tile_pool(name="p_mb", bufs=1)
    mb = p_mb.tile([P, RT, T], FP8, name="mb")

    # ---------------- Stage A: per-tile LN1 -> transpose -> fp8 cast -> V proj
    p_wv = tc.alloc_tile_pool(name="p_wv", bufs=1)
    wv_t = p_wv.tile([P, CT, C], FP8, name="wv_t")
    lnA = tc.alloc_tile_pool(name="lnA", bufs=1)
    bv_bc = _bcast_load(nc, lnA, d["bv"], "bv_bc", BF16)
    pA = tc.alloc_tile_pool(name="pA", bufs=2)
    psA = tc.alloc_tile_pool(name="psA", bufs=2, space="PSUM")

    # Ring discipline: the sync ring carries ONLY the latency-critical x tiles
    # (and later the transposes + final stores). All bulk streams on gpsimd in
    # explicit priority order; pool-slot rotation paces the Q/K weight stream
    # one head ahead of compute. The scalar ring stays clear for the stage-A
    # transposes, then carries half the w1/w2 streams.
    xts = []
    for tt in range(TT):
        xt = pA.tile([P, C], BF16, name=f"xt{tt}", tag="xt", bufs=2)
        nc.sync.dma_start(xt[:], d["xb"][tt * P:(tt + 1) * P, :])
        xts.append(xt[:])
    for kc in range(4):
        nc.gpsimd.dma_start(wv_t[:, 4 * kc:4 * (kc + 1), :],
                            d["wv"][:, 4 * kc:4 * (kc + 1), :])
    nc.gpsimd.dma_start(out=mb[:], in_=d["mask"].rearrange("(qo qp) k -> qp qo k", qp=P))
    bq_t = consts.tile([P, CT], F32, name="bq_t")
    nc.gpsimd.dma_start(out=bq_t[:], in_=d["bq"])
    bk_t = consts.tile([P, CT], F32, name="bk_t")
    nc.gpsimd.dma_start(out=bk_t[:], in_=d["bk"])
    b1_t = consts.tile([P, FT], F32, name="b1_t")
    nc.gpsimd.dma_start(out=b1_t[:], in_=d["b1"])
    # Gate the bulk weight stream behind the last lo-tile cast: the stage-A
    # critical path (x tiles + wv + the LN transposes' descriptor storms) gets
    # the DMA engines to itself for the first ~25us. The copy blocks the
    # gpsimd FIFO until xnT8_lo is fully written.
    gate_t = consts.tile([P, 1], F32, name="gate_t")
    gate_src = None  # set inside the stage-A loop below

    def emit_bulk_dmas():
        nc.gpsimd.tensor_copy(gate_t[:], gate_src)
        for h in range(H):
            for fl in range(HT):
                fo = h * HT + fl
                wqc = pQKw.tile([P, CT, P], FP8, name=f"wqc{fo}", tag="wqc", bufs=4)
                nc.gpsimd.dma_start(wqc[:], d["wq"][fo])
                wqcs[fo] = wqc
                wkc = pQKw.tile([P, CT, P], FP8, name=f"wkc{fo}", tag="wkc", bufs=4)
                nc.gpsimd.dma_start(wkc[:], d["wk"][fo])
                wkcs[fo] = wkc
        # wo for stage D, then the f32 residual rows (bo folded in on host),
        # all behind the (self-pacing) qk stream.
        for kc in range(4):
            nc.gpsimd.dma_start(wo_t[:, 4 * kc:4 * (kc + 1), :, :],
                                d["wo"][:, 4 * kc:4 * (kc + 1), :, :])
        nc.gpsimd.dma_start(out=r_sb[:],
                            in_=d["xq"].rearrange("(qo qp) c -> qp qo c", qp=P))

    wqcs, wkcs = {}, {}
    # preload the gelu activation table off the D->F critical path
    gdum = consts.tile([P, 1], F32, name="gdum")
    nc.scalar.activation(out=gdum[:], in_=eps_t[:], func=ACT.Gelu,
                         bias=0.0, scale=1.0)

    # software-pipelined by one tile: cast(tt)+V(tt) are emitted during
    # LN(tt+1) so the fp8 cast's transpose-wait never blocks the next LN
    # apply in the scalar FIFO.
    xnTts = {}

    def _emit_castv(tt):
        half, lt = divmod(tt, 4)
        nc.vector.tensor_copy(xnT8[half][:, :, lt * P:(lt + 1) * P],
                              xnTts.pop(tt)[:])
        for h in range(H):
            ps_v = psA.tile([P, DH], F32, name="ps_v", tag="psA", bufs=2)
            for kp in range(CP):
                nc.tensor.matmul(ps_v[:],
                                 xnT8[half][:, 2 * kp:2 * kp + 2, lt * P:(lt + 1) * P],
                                 wv_t[:, 2 * kp:2 * kp + 2, h * DH:(h + 1) * DH],
                                 start=(kp == 0), stop=(kp == CP - 1), perf_mode=DR)
            nc.vector.tensor_tensor(vh[:, tt, h * DH:(h + 1) * DH], ps_v[:],
                                    bv_bc[:, h * DH:(h + 1) * DH], OP.add)

    for tt in range(TT):
        xn_t = _ln_tile(nc, pA, xts[tt], eps_t, "ln1", tt)
        xnTt = pA.tile([P, CT, P], BF16, name=f"xnTt{tt}", tag="xnTt", bufs=2)
        nc.scalar.dma_start_transpose(xnTt[:], xn_t[:])
        xnTts[tt] = xnTt
        if tt > 0:
            _emit_castv(tt - 1)
        if tt == 4:  # cast(t3) just emitted -> xnT8_lo complete
            gate_src = xnT8_lo[:, 0:1, 3 * P:3 * P + 1]
            emit_bulk_dmas()
    _emit_castv(TT - 1)
    psA.release()
    pA.release()
    lnA.release()
    p_wv.release()

    # ---------------- Stage B+C: software-pipelined per-head Q/K + attention
    pBC = tc.alloc_tile_pool(name="pBC", bufs=2)
    psBC = tc.alloc_tile_pool(name="psBC", bufs=2, space="PSUM")
    hs = {}

    def emit_qk(h):
        qTh = pBC.tile([P, HT, R], FP8, name=f"qTh{h}", tag="qTh", bufs=2)
        kTh = pBC.tile([P, HT, T], FP8, name=f"kTh{h}", tag="kTh", bufs=2)
        for fl in range(HT):
            fo = h * HT + fl
            wqc, wkc = wqcs.pop(fo), wkcs.pop(fo)
            ps_q = psBC.tile([P, R], F32, name="ps_q", tag="psB", bufs=2)
            for kp in range(CP):
                nc.tensor.matmul(ps_q[:], wqc[:, 2 * kp:2 * kp + 2, :],
                                 xnT8_lo[:, 2 * kp:2 * kp + 2, :],
                                 start=(kp == 0), stop=(kp == CP - 1), perf_mode=DR)
            nc.scalar.activation(out=qTh[:, fl, :], in_=ps_q[:], func=ACT.Identity,
                                 bias=bq_t[:, fo:fo + 1], scale=1.0)
            for nn in range(2):
                ps_k = psBC.tile([P, 512], F32, name="ps_k", tag="psB", bufs=2)
                for kp in range(CP):
                    nc.tensor.matmul(ps_k[:], wkc[:, 2 * kp:2 * kp + 2, :],
                                     xnT8[nn][:, 2 * kp:2 * kp + 2, :],
                                     start=(kp == 0), stop=(kp == CP - 1), perf_mode=DR)
                nc.scalar.activation(out=kTh[:, fl, nn * 512:(nn + 1) * 512], in_=ps_k[:],
                                     func=ACT.Identity, bias=bk_t[:, fo:fo + 1],
                                     scale=1.0)
        hs[h] = (qTh, kTh)

    def emit_scores(h):
        qTh, kTh = hs[h]
        attT = pBC.tile([P, TT, R], BF16, name=f"attT{h}", tag="attT", bufs=1)
        attT8 = pBC.tile([P, TT, R], FP8, name=f"attT8{h}", tag="attT8", bufs=2)
        for qt in range(RT):
            ps_s = psBC.tile([P, T], F32, name="ps_s", tag="scores", bufs=2)
            for nn in range(2):
                for dp in range(HT // 2):
                    nc.tensor.matmul(
                        ps_s[:, nn * 512:(nn + 1) * 512],
                        qTh[:, 2 * dp:2 * dp + 2, qt * P:(qt + 1) * P],
                        kTh[:, 2 * dp:2 * dp + 2, nn * 512:(nn + 1) * 512],
                        start=(dp == 0), stop=(dp == HT // 2 - 1), perf_mode=DR)
            s_sb = pBC.tile([P, T], F32, name="s_sb", tag="s_sb", bufs=1)
            nc.vector.scalar_tensor_tensor(s_sb[:], ps_s[:], ISQ, mb[:, qt, :],
                                           OP.mult, OP.add)
            # logits are bounded (<= ~15) so exp needs no max-subtraction
            e_sb = pBC.tile([P, T], BF16, name="e_sb", tag="e_sb", bufs=2)
            sums = pBC.tile([P, 1], F32, name="sums", tag="sums", bufs=2)
            nc.scalar.activation(out=e_sb[:], in_=s_sb[:], func=ACT.Exp,
                                 bias=0.0, scale=1.0, accum_out=sums[:])
            recip = pBC.tile([P, 1], F32, name="recip", tag="recip", bufs=2)
            nc.vector.reciprocal(recip[:], sums[:])
            nc.vector.tensor_scalar_mul(e_sb[:], e_sb[:], recip[:])
            nc.sync.dma_start_transpose(attT[:, :, qt * P:(qt + 1) * P], e_sb[:])
        if h == H - 1:  # per-qt cast so the split AV can start immediately
            for qt in range(RT):
                nc.vector.tensor_copy(attT8[:, :, qt * P:(qt + 1) * P],
                                      attT[:, :, qt * P:(qt + 1) * P])
        else:
            nc.vector.tensor_copy(attT8[:], attT[:])
        hs[h] = hs[h] + (attT8,)

    def emit_av(h):
        _, _, attT8 = hs.pop(h)
        nq = RT if h == H - 1 else 1   # last head: split over qt chunks so AV
        nw = R // nq                   # overlaps the tail softmax chain
        for dt_ in range(HT):
            ps_y = psBC.tile([P, R], F32, name="ps_y", tag="av", bufs=2)
            for qc in range(nq):
                for kp in range(TT // 2):
                    nc.tensor.matmul(
                        ps_y[:, qc * nw:(qc + 1) * nw],
                        vh[:, 2 * kp:2 * kp + 2, h * DH + dt_ * P:h * DH + (dt_ + 1) * P],
                        attT8[:, 2 * kp:2 * kp + 2, qc * nw:(qc + 1) * nw],
                        start=(kp == 0), stop=(kp == TT // 2 - 1), perf_mode=DR)
            nc.scalar.activation(out=yT[:, h * HT + dt_, :], in_=ps_y[:], func=ACT.Copy)

    emit_qk(0)
    emit_scores(0)
    # prefetch the first w1 chunks (scalar/sync queues) during attention
    w1cs = {}
    for fp in range(3):
        w1c = pW1.tile([P, CT, 2, P], BF16, name="w1c", tag="w1c", bufs=3)
        (nc.scalar if fp % 2 == 0 else nc.gpsimd).dma_start(w1c[:], d["w1"][fp])
        w1cs[fp] = w1c
    for h in range(H):
        if h + 1 < H:
            emit_qk(h + 1)
        emit_av(h)
        if h + 1 < H:
            emit_scores(h + 1)
    psBC.release()
    pBC.release()
    p_mb.release()
    pQKw.release()
    p_vh.release()
    p_xnT.release()

    # ones1 has a single 1 in row 0: ones1.T @ bias_bc adds a bias chunk
    # inside the matmul accumulation (used in stages D and G).
    ones1 = consts.tile([P, P], BF16, name="ones1")
    nc.vector.memset(ones1[:], 0.0)
    nc.vector.memset(ones1[0:1, :], 1.0)

    # ---------------- Stage D: per-qt O-proj + residual (in SBUF) + LN2
    p_hT = tc.alloc_tile_pool(name="p_hT", bufs=1)
    hT = p_hT.tile([P, FT, R], BF16, name="hT")
    p_xn2T = tc.alloc_tile_pool(name="p_xn2T", bufs=1)
    xn2T = p_xn2T.tile([P, CT, R], BF16, name="xn2T")
    psX = tc.alloc_tile_pool(name="psX", bufs=8, space="PSUM")
    pD = tc.alloc_tile_pool(name="pD", bufs=2)
    for qt in range(RT):
        for fc in range(4):
            ps_o = psX.tile([P, 512], F32, name="ps_o", tag="ps", bufs=8)
            for kp in range(CP):
                nc.tensor.matmul(ps_o[:], yT[:, 2 * kp:2 * kp + 2, qt * P:(qt + 1) * P],
                                 wo_t[:, 2 * kp:2 * kp + 2, fc, :],
                                 start=(kp == 0), stop=(kp == CP - 1), perf_mode=DR)
            r_sl = r_sb[:, qt, fc * 512:(fc + 1) * 512]
            nc.vector.tensor_tensor(r_sl, ps_o[:], r_sl, OP.add)
        xn2_t = _ln_tile(nc, pD, r_sb[:, qt, :], eps_t, "ln2", qt, xh_bufs=2)
        nc.sync.dma_start_transpose(xn2T[:, :, qt * P:(qt + 1) * P], xn2_t[:])

    # ---------------- Stage F: MLP up + gelu -> hT [128, FT, R] bf16
    # First 3 chunks run as two FD-256 halves (gated on xn2T qt01 / qt23) so
    # the PE starts MLP-up before the last LN2 tiles finish.
    for fp in range(FT // 2):
        if fp in w1cs:
            w1c = w1cs.pop(fp)
        else:
            w1c = pW1.tile([P, CT, 2, P], BF16, name="w1c", tag="w1c", bufs=3)
            (nc.scalar if fp % 2 == 0 else nc.gpsimd).dma_start(w1c[:], d["w1"][fp])
        for fl in range(2):
            fo = 2 * fp + fl
            ps_h = psX.tile([P, R], F32, name="ps_h", tag="ps", bufs=8)
            if fp < 3:
                for hf in range(2):
                    sl = slice(hf * 256, (hf + 1) * 256)
                    for ki in range(CT):
                        nc.tensor.matmul(ps_h[:, sl], w1c[:, ki, fl, :],
                                         xn2T[:, ki, sl],
                                         start=(ki == 0), stop=(ki == CT - 1))
            else:
                for ki in range(CT):
                    nc.tensor.matmul(ps_h[:], w1c[:, ki, fl, :], xn2T[:, ki, :],
                                     start=(ki == 0), stop=(ki == CT - 1))
            nc.scalar.activation(out=hT[:, fo, :], in_=ps_h[:], func=ACT.Gelu,
                                 bias=b1_t[:, fo:fo + 1], scale=1.0)
        if fp == 2:  # yT/wo are dead once stage D's matmuls retire
            p_yT.release()
            p_wo.release()

    # ---------------- Stage G: MLP down + residual -> out
    pG = tc.alloc_tile_pool(name="pG", bufs=2)
    b2_bc = _bcast_load(nc, pG, d["b2"], "b2_bc", BF16)
    for fc in range(4):
        ps4 = [psX.tile([P, 512], F32, name=f"ps_g{qt}", tag="ps", bufs=8)
               for qt in range(RT)]
        for hb in range(FT // 8):
            w2b = pG.tile([P, 8, 512], BF16, name="w2b", tag="w2b", bufs=3)
            (nc.gpsimd if hb % 2 == 0 else nc.scalar).dma_start(w2b[:], d["w2"][fc, hb])
            for hl in range(8):
                ho = hb * 8 + hl
                for qt in range(RT):
                    nc.tensor.matmul(ps4[qt][:], hT[:, ho, qt * P:(qt + 1) * P],
                                     w2b[:, hl, :], start=(ho == 0), stop=False)
        for qt in range(RT):
            nc.tensor.matmul(ps4[qt][:], ones1[:], b2_bc[:, fc * 512:(fc + 1) * 512],
                             start=False, stop=True)
            o_t = pG.tile([P, 512], F32, name="o_t", tag="o_t", bufs=3)
            nc.vector.tensor_tensor(o_t[:], ps4[qt][:],
                                    r_sb[:, qt, fc * 512:(fc + 1) * 512], OP.add)
            nc.sync.dma_start(d["out"][qt * P:(qt + 1) * P, fc * 512:(fc + 1) * 512],
                                  o_t[:])
    pG.release()
    pD.release()
    psX.release()
    p_xn2T.release()
    p_hT.release()
    pW1.release()
    p_r.release()
    consts.release()


def build_program():
    nc = bacc.Bacc("TRN2", target_bir_lowering=False, debug=False, num_devices=8)
    with tile.TileContext(nc) as tc:
        _body(tc)
    nc.compile()
    return nc


_prog = None


def _get_prog():
    global _prog
    if _prog is None:
        _prog = build_program()
    return _prog


def make_in_maps(x, mask, Wq, bq, Wk, bk, Wv, bv, Wo, bo,
                 ln1_w, ln1_b, ln2_w, ln2_b, W1, b1, W2, b2):
    bf = ml_dtypes.bfloat16
    f8 = ml_dtypes.float8_e4m3
    f32 = np.float32
    cc = np.ascontiguousarray

    def f(a):
        return np.asarray(a, dtype=f32)

    x, mask = np.asarray(x, dtype=f32), np.asarray(mask)
    # fold the LN affines into the consuming matmuls: for y = ln(x)@W + b with
    # ln(x) = z*w + b_ln (z the normalized input), y = z@(w[:,None]*W) + (b_ln@W + b)
    w1l, b1l = f(ln1_w)[:, None], f(ln1_b)
    w2l, b2l = f(ln2_w)[:, None], f(ln2_b)
    Wq_, Wk_, Wv_, W1_ = w1l * f(Wq), w1l * f(Wk), w1l * f(Wv), w2l * f(W1)
    bq_, bk_ = f(bq) + b1l @ f(Wq), f(bk) + b1l @ f(Wk)
    bv_, b1_ = f(bv) + b1l @ f(Wv), f(b1) + b2l @ f(W1)
    wq_h = cc(Wq_.astype(f8).reshape(CT, P, CT, P).transpose(2, 1, 0, 3))
    wk_h = cc(Wk_.astype(f8).reshape(CT, P, CT, P).transpose(2, 1, 0, 3))
    wv_h = cc(Wv_.astype(f8).reshape(CT, P, C).transpose(1, 0, 2))
    wo_h = cc(f(Wo).astype(f8).reshape(CT, P, 4, 512).transpose(1, 0, 2, 3))
    w1_h = cc(W1_.astype(bf).reshape(CT, P, FT // 2, 2, P).transpose(2, 1, 0, 3, 4))
    w2_h = cc(f(W2).astype(bf).reshape(FT // 8, 8, P, 4, 512).transpose(3, 0, 2, 1, 4))
    shared = dict(
        wq=wq_h, wk=wk_h, wv=wv_h, wo=wo_h, w1=w1_h, w2=w2_h,
        bq=cc(bq_.reshape(CT, P).T), bk=cc(bk_.reshape(CT, P).T),
        b1=cc(b1_.reshape(FT, P).T),
        bv=bv_.astype(bf), b2=f(b2).astype(bf),
    )
    in_maps = []
    bo32 = f(bo)
    for c in range(8):
        b, hh = divmod(c, 2)
        xc = np.roll(x[b], -hh * R, axis=0)
        mk = np.roll((np.asarray(mask[b, hh * R:(hh + 1) * R, :], dtype=f32) - 1.0)
                     * 240.0, -hh * R, axis=1).astype(f8)
        in_maps.append({**shared, "xb": cc(xc.astype(bf)),
                        "xq": cc(xc[:R] + bo32), "mask": cc(mk)})
    return in_maps


def kernel(**inputs):
    nc = _get_prog()
    in_maps = make_in_maps(**inputs)
    res = run_bass_kernel_spmd(nc, in_maps, core_ids=list(range(8)))
    out = np.empty((B, T, C), np.float32)
    for c in range(8):
        b, hh = divmod(c, 2)
        out[b, hh * R:(hh + 1) * R, :] = res.results[c]["out"]
    return out
